# revision 2
# baseline (speedup 1.0000x reference)
"""Trainium2 Bass kernel for nn_Attention (non-local-block attention), v2.

Reference (per batch b, z flattened to [Ci=128, N=4096]):
    theta = w_theta @ z + b_theta        [64, N]
    phi   = w_phi   @ z + b_phi          [64, N]
    psi   = w_psi   @ z + b_psi          [64, N]
    G     = relu((phi^T psi) / N)        [N, N]
    out   = w_v @ (G @ theta^T)^T + b_v + z

Key algebraic restructure: fold w_v into theta.
    WVTH = w_v @ theta = WTH @ z + bvth,  WTH = w_v @ w_theta (host),
    bvth = w_v @ b_theta (host).
    out[ci,n] = (1/N) * sum_m relu(g)[n,m] * WVTH[ci,m] + b_v[ci] + z[ci,n]

Sharding: 8 cores = 2 batches x 4 token-blocks of 1024. Each core computes
out[b][:, blk]. The core's zb16 is host-permuted in 1024-col chunks so its own
block comes first (phi available earliest); m is summed over, so chunk order
does not affect the result.

Per-core dataflow (m-tile = 128 tokens, 32 m-tiles):
  psi [64,4096] bf16, phi [64,1024] bf16: projections (chunk0 stacked
      [wpsi|wphi] stationary), biases via activation-drain.
  wvthT [128m,128ci] per m-tile: z-chunk-stationary matmuls, + b_theta folded
      via w_v (bvth) added by a broadcast-tile STT drain; stored fp8e4.
  vg PSUM [128,1024] f32 prefilled with 4096*z (identity matmul) then
      accumulates sum_m s * wvthT via fp8 DoubleRow (2 m-tiles per pass).
  main loop per m-tile pair: gT = psi^T phi (K=64 bf16, 4 MMs of 512),
      relu -> fp8 s pair (even tile on ScalarE, odd on VectorE, concurrent),
      vg DoubleRow MMs lagged 2 pairs.
  tail: out = vg_psum * (1/4096) + b_v, split ScalarE/VectorE, 2 DMAs.
"""

import ml_dtypes
import numpy as np

import concourse.bacc as bacc
import concourse.mybir as mybir
import concourse.tile as tile
from concourse.bass_utils import run_bass_kernel_spmd

F32 = mybir.dt.float32
BF16 = mybir.dt.bfloat16
FP8 = mybir.dt.float8e4
AF = mybir.ActivationFunctionType
ALU = mybir.AluOpType
DR = mybir.MatmulPerfMode.DoubleRow
BF16NP = ml_dtypes.bfloat16

B, CI, CO = 2, 128, 64
T, H, W = 4, 32, 32
N = T * H * W            # 4096 tokens
NCORES = 8
BLK = N // (NCORES // B)  # 1024 tokens per core
MT = N // 128            # 32 m-tiles
NCHUNK = 4               # zb16 DMA chunks of 1024 cols
SCALE = float(1.0 / N)

_CACHE = {}


def _build():
    nc = bacc.Bacc("TRN2", target_bir_lowering=False, debug=False)

    wpack = nc.dram_tensor("wpack", [CI, 384], BF16, kind="ExternalInput")
    biaspack = nc.dram_tensor("biaspack", [CI, 4], F32, kind="ExternalInput")
    bvthbc = nc.dram_tensor("bvthbc", [CI, 1024], BF16, kind="ExternalInput")
    zb16 = nc.dram_tensor("zb16", [CI, N], BF16, kind="ExternalInput")
    out = nc.dram_tensor("out", [CI, BLK], F32, kind="ExternalOutput")

    with tile.TileContext(nc) as tc:
        with (
            tc.tile_pool(name="const", bufs=1) as cpool,
            tc.tile_pool(name="zp", bufs=1) as zp,
            tc.tile_pool(name="proj", bufs=1) as pp,
            tc.tile_pool(name="s8p", bufs=3) as sp,
            tc.tile_pool(name="tail", bufs=1) as tailp,
            tc.tile_pool(name="vgps", bufs=1, space="PSUM") as vgpool,
        ):
            # ---- input DMAs, in consumption order, on the sync queue ----
            wpack_sb = cpool.tile([CI, 384], BF16)
            nc.sync.dma_start(wpack_sb[:], wpack[:])
            biaspack_sb = cpool.tile([CI, 4], F32)
            nc.sync.dma_start(biaspack_sb[:], biaspack[:])
            bvthbc_sb = cpool.tile([CI, 1024], BF16)
            nc.sync.dma_start(bvthbc_sb[:], bvthbc[:])
            zb16_sb = zp.tile([CI, N], BF16)
            for c in range(NCHUNK):
                nc.sync.dma_start(
                    zb16_sb[:, c * 1024:(c + 1) * 1024],
                    zb16[:, c * 1024:(c + 1) * 1024],
                )

            wpsi_sb = wpack_sb[:, 0:64]
            wphi_sb = wpack_sb[:, 64:128]
            wpsiphi_sb = wpack_sb[:, 0:128]     # warmup operand only
            wtht_sb = wpack_sb[:, 128:256]      # (w_v @ w_theta).T
            ident_sb = wpack_sb[:, 256:384]     # 4096 * I
            bpsi_sb = biaspack_sb[0:CO, 0:1]
            bphi_sb = biaspack_sb[0:CO, 2:3]
            bv_sb = biaspack_sb[:, 1:2]

            psi_sb = pp.tile([CO, N], BF16)
            phi_sb = pp.tile([CO, BLK], BF16)
            wvth8_sb = pp.tile([CI, MT, CI], FP8)

            # vg accumulator: 2 PSUM banks, lives for the whole kernel
            vg_ps = vgpool.tile([CI, BLK], F32)

            # ---- HAM ignition: dummy matmuls while DMAs land ----
            with tc.tile_pool(name="warm", bufs=1, space="PSUM") as wpool:
                wps = wpool.tile([CI, 512], F32)
                for _ in range(10):
                    nc.tensor.matmul(
                        wps[:, 0:128], wpsiphi_sb, wpsiphi_sb,
                        skip_group_check=True,
                    )

            # ---- residual prefill: vg_ps = 4096 * z(block) ----
            for h in range(2):
                nc.tensor.matmul(
                    vg_ps[:, h * 512:(h + 1) * 512],
                    ident_sb,
                    zb16_sb[:, h * 512:(h + 1) * 512],
                    start=True, stop=False, skip_group_check=True,
                )

            # ---- projection phase: psi/phi + wvthT, chunk by chunk ----
            with tc.tile_pool(name="pjps", bufs=2, space="PSUM") as pjps:
                for c in range(NCHUNK):
                    base = c * 1024
                    # psi projection (ScalarE drains); chunk0 also phi (VectorE)
                    ps = pjps.tile([CI, 1024], F32, tag="pj", name=f"pj{c}")[0:CO, :]
                    for h in range(2):
                        nc.tensor.matmul(
                            ps[:, h * 512:(h + 1) * 512],
                            wpsi_sb,
                            zb16_sb[:, base + h * 512:base + (h + 1) * 512],
                        )
                    nc.scalar.activation(
                        psi_sb[:, base:base + 1024], ps[:],
                        AF.Identity, bias=bpsi_sb,
                    )
                    if c == 0:
                        php = pjps.tile([CI, 1024], F32, tag="pj", name="phj")[0:CO, :]
                        for h in range(2):
                            nc.tensor.matmul(
                                php[:, h * 512:(h + 1) * 512],
                                wphi_sb,
                                zb16_sb[:, h * 512:(h + 1) * 512],
                            )
                        nc.vector.tensor_scalar_add(phi_sb[:], php[:], bphi_sb)
                    # wvthT for the chunk's 8 m-tiles, 2 banks
                    wv_ps = pjps.tile([CI, 1024], F32, tag="pj", name=f"wv{c}")
                    for j in range(8):
                        mt = c * 8 + j
                        nc.tensor.matmul(
                            wv_ps[:, j * 128:(j + 1) * 128],
                            zb16_sb[:, mt * 128:(mt + 1) * 128],
                            wtht_sb,
                            start=(j % 4 == 0), stop=(j % 4 == 3),
                            skip_group_check=True,
                        )
                    # + bvth (varies along ci columns): broadcast-tile add
                    nc.vector.scalar_tensor_tensor(
                        wvth8_sb[:, c * 8:(c + 1) * 8, :],
                        wv_ps[:],
                        1.0,
                        bvthbc_sb[:],
                        ALU.mult,
                        ALU.add,
                    )

            # ---- main loop: 16 m-tile pairs ----
            s8 = {}
            g = {}

            def emit_g(mt, gpool):
                gt = gpool.tile([CI, 1024], F32, tag="g", name=f"g{mt}")
                msl = slice(mt * 128, (mt + 1) * 128)
                for h in range(2):
                    nc.tensor.matmul(
                        gt[:, h * 512:(h + 1) * 512],
                        psi_sb[:, msl],
                        phi_sb[:, h * 512:(h + 1) * 512],
                    )
                g[mt] = gt

            def emit_relu(p):
                s = sp.tile([CI, 2, 1024], FP8, tag="s", name=f"s{p}")
                nc.scalar.activation(s[:, 0, :], g.pop(2 * p)[:], AF.Relu)
                nc.vector.tensor_scalar_max(s[:, 1, :], g.pop(2 * p + 1)[:], 0.0)
                s8[p] = s

            def emit_vg(p):
                s = s8.pop(p)
                for h in range(2):
                    nc.tensor.matmul(
                        vg_ps[:, h * 512:(h + 1) * 512],
                        wvth8_sb[:, 2 * p:2 * p + 2, :],
                        s[:, :, h * 512:(h + 1) * 512],
                        start=False, stop=(p == MT // 2 - 1),
                        perf_mode=DR, skip_group_check=True,
                    )

            with tc.tile_pool(name="gps", bufs=3, space="PSUM") as gpool:
                for p in range(MT // 2):
                    emit_g(2 * p, gpool)
                    emit_g(2 * p + 1, gpool)
                    if p >= 2:
                        emit_vg(p - 2)
                    emit_relu(p)
                emit_vg(MT // 2 - 2)
                emit_vg(MT // 2 - 1)

            # ---- tail: out = vg/4096 + b_v ----
            out0 = tailp.tile([CI, 512], F32, name="out0")
            out1 = tailp.tile([CI, 512], F32, name="out1")
            nc.scalar.activation(
                out0[:], vg_ps[:, 0:512], AF.Identity, bias=bv_sb, scale=SCALE
            )
            nc.vector.tensor_scalar(
                out1[:], vg_ps[:, 512:1024], SCALE, bv_sb, ALU.mult, ALU.add
            )
            nc.sync.dma_start(out[:, 0:512], out0[:])
            nc.sync.dma_start(out[:, 512:1024], out1[:])

    nc.compile()
    return nc


def _get_nc():
    if "nc" not in _CACHE:
        _CACHE["nc"] = _build()
    return _CACHE["nc"]


def build_in_maps(z, w_theta, b_theta, w_phi, b_phi, w_psi, b_psi, w_v, b_v):
    z2 = np.asarray(z, np.float32).reshape(B, CI, N)
    z216 = z2.astype(BF16NP)

    wpsiT = np.asarray(w_psi, np.float32).T          # [128, 64]
    wphiT = np.asarray(w_phi, np.float32).T
    wv = np.asarray(w_v, np.float32)                 # [128, 64]
    wth = wv @ np.asarray(w_theta, np.float32)       # [128, 128]
    wpack = np.ascontiguousarray(
        np.concatenate(
            [wpsiT, wphiT, wth.T, np.float32(N) * np.eye(CI, dtype=np.float32)],
            axis=1,
        ).astype(BF16NP)
    )
    biaspack = np.stack(
        [
            np.concatenate([b_psi, b_phi]).astype(np.float32),
            np.asarray(b_v, np.float32),
            np.zeros(CI, np.float32),
            np.zeros(CI, np.float32),
        ],
        axis=1,
    ).astype(np.float32)
    bvth = wv @ np.asarray(b_theta, np.float32)      # [128]
    bvthbc = np.ascontiguousarray(
        np.broadcast_to(np.tile(bvth, 8), (CI, 1024)).astype(BF16NP)
    )

    in_maps = []
    for core in range(NCORES):
        b, nb = divmod(core, NCORES // B)
        chunks = [nb] + [c for c in range(NCHUNK) if c != nb]
        zperm = np.concatenate(
            [z216[b][:, c * 1024:(c + 1) * 1024] for c in chunks], axis=1
        )
        in_maps.append(
            {
                "wpack": wpack,
                "biaspack": biaspack,
                "bvthbc": bvthbc,
                "zb16": np.ascontiguousarray(zperm),
            }
        )
    return in_maps


def kernel(z, w_theta, b_theta, w_phi, b_phi, w_psi, b_psi, w_v, b_v):
    in_maps = build_in_maps(
        z, w_theta, b_theta, w_phi, b_phi, w_psi, b_psi, w_v, b_v
    )
    nc = _get_nc()
    res = run_bass_kernel_spmd(nc, in_maps, core_ids=list(range(NCORES)))

    out_full = np.empty((B, CI, N), dtype=np.float32)
    for core in range(NCORES):
        b, nb = divmod(core, NCORES // B)
        out_full[b][:, nb * BLK:(nb + 1) * BLK] = res.results[core]["out"]
    return out_full.reshape(B, CI, T, H, W)


# revision 3
# speedup vs baseline: 1.4121x; 1.4121x over previous
"""Trainium2 Bass kernel for nn_Attention (non-local-block attention), v2.

Reference (per batch b, z flattened to [Ci=128, N=4096]):
    theta = w_theta @ z + b_theta        [64, N]
    phi   = w_phi   @ z + b_phi          [64, N]
    psi   = w_psi   @ z + b_psi          [64, N]
    G     = relu((phi^T psi) / N)        [N, N]
    out   = w_v @ (G @ theta^T)^T + b_v + z

Key algebraic restructure: fold w_v into theta.
    WVTH = w_v @ theta = WTH @ z + bvth,  WTH = w_v @ w_theta (host),
    bvth = w_v @ b_theta (host).
    out[ci,n] = (1/N) * sum_m relu(g)[n,m] * WVTH[ci,m] + b_v[ci] + z[ci,n]

Sharding: 8 cores = 2 batches x 4 token-blocks of 1024. Each core computes
out[b][:, blk]. The core's zb16 is host-permuted in 1024-col chunks so its own
block comes first (phi available earliest); m is summed over, so chunk order
does not affect the result.

Per-core dataflow (m-tile = 128 tokens, 32 m-tiles):
  psi [64,4096] bf16, phi [64,1024] bf16: projections (chunk0 stacked
      [wpsi|wphi] stationary), biases via activation-drain.
  wvthT [128m,128ci] per m-tile: z-chunk-stationary matmuls, + b_theta folded
      via w_v (bvth) added by a broadcast-tile STT drain; stored fp8e4.
  vg PSUM [128,1024] f32 prefilled with 4096*z (identity matmul) then
      accumulates sum_m s * wvthT via fp8 DoubleRow (2 m-tiles per pass).
  main loop per m-tile pair: gT = psi^T phi (K=64 bf16, 4 MMs of 512),
      relu -> fp8 s pair (even tile on ScalarE, odd on VectorE, concurrent),
      vg DoubleRow MMs lagged 2 pairs.
  tail: out = vg_psum * (1/4096) + b_v, split ScalarE/VectorE, 2 DMAs.
"""

import ml_dtypes
import numpy as np

import concourse.bacc as bacc
import concourse.mybir as mybir
import concourse.tile as tile
from concourse.bass_utils import run_bass_kernel_spmd

F32 = mybir.dt.float32
BF16 = mybir.dt.bfloat16
FP8 = mybir.dt.float8e4
AF = mybir.ActivationFunctionType
ALU = mybir.AluOpType
DR = mybir.MatmulPerfMode.DoubleRow
BF16NP = ml_dtypes.bfloat16

B, CI, CO = 2, 128, 64
T, H, W = 4, 32, 32
N = T * H * W            # 4096 tokens
NCORES = 8
BLK = N // (NCORES // B)  # 1024 tokens per core
MT = N // 128            # 32 m-tiles
NCHUNK = 4               # zb16 DMA chunks of 1024 cols
SCALE = float(1.0 / N)

_CACHE = {}


def _build():
    nc = bacc.Bacc("TRN2", target_bir_lowering=False, debug=False)

    wpack = nc.dram_tensor("wpack", [CI, 384], BF16, kind="ExternalInput")
    biaspack = nc.dram_tensor("biaspack", [CI, 4], F32, kind="ExternalInput")
    bvthbc = nc.dram_tensor("bvthbc", [CI, 1024], BF16, kind="ExternalInput")
    zb16 = nc.dram_tensor("zb16", [CI, N], BF16, kind="ExternalInput")
    out = nc.dram_tensor("out", [CI, BLK], F32, kind="ExternalOutput")

    with tile.TileContext(nc) as tc:
        with (
            tc.tile_pool(name="const", bufs=1) as cpool,
            tc.tile_pool(name="zp", bufs=1) as zp,
            tc.tile_pool(name="proj", bufs=1) as pp,
            tc.tile_pool(name="s8p", bufs=3) as sp,
            tc.tile_pool(name="tail", bufs=1) as tailp,
            tc.tile_pool(name="vgps", bufs=1, space="PSUM") as vgpool,
        ):
            # ---- input DMAs, in consumption order, on the sync queue ----
            wpack_sb = cpool.tile([CI, 384], BF16)
            nc.sync.dma_start(wpack_sb[:], wpack[:])
            biaspack_sb = cpool.tile([CI, 4], F32)
            nc.sync.dma_start(biaspack_sb[:], biaspack[:])
            bvthbc_sb = cpool.tile([CI, 1024], BF16)
            nc.sync.dma_start(bvthbc_sb[:], bvthbc[:])
            zb16_sb = zp.tile([CI, N], BF16)
            for c in range(NCHUNK):
                nc.sync.dma_start(
                    zb16_sb[:, c * 1024:(c + 1) * 1024],
                    zb16[:, c * 1024:(c + 1) * 1024],
                )

            wpsi_sb = wpack_sb[:, 0:64]
            wphi_sb = wpack_sb[:, 64:128]
            wpsiphi_sb = wpack_sb[:, 0:128]     # warmup operand only
            wtht_sb = wpack_sb[:, 128:256]      # (w_v @ w_theta).T
            ident_sb = wpack_sb[:, 256:384]     # 4096 * I
            bpsi_sb = biaspack_sb[0:CO, 0:1]
            bphi_sb = biaspack_sb[0:CO, 2:3]
            bv_sb = biaspack_sb[:, 1:2]

            psi_sb = pp.tile([CO, N], BF16)
            phi_sb = pp.tile([CO, BLK], BF16)
            wvth8_sb = pp.tile([CI, MT, CI], FP8)

            # vg accumulator: 2 PSUM banks, lives for the whole kernel
            vg_ps = vgpool.tile([CI, BLK], F32)

            # ---- HAM ignition: a short dummy burst while DMAs land ----
            with tc.tile_pool(name="warm", bufs=1, space="PSUM") as wpool:
                wps = wpool.tile([CI, 512], F32)
                for _ in range(4):
                    nc.tensor.matmul(
                        wps[:, 0:384], wpsiphi_sb, wpack_sb[:, 0:384],
                        skip_group_check=True,
                    )

            # ---- residual prefill: vg_ps = 4096 * z(block) ----
            for h in range(2):
                nc.tensor.matmul(
                    vg_ps[:, h * 512:(h + 1) * 512],
                    ident_sb,
                    zb16_sb[:, h * 512:(h + 1) * 512],
                    start=True, stop=False, skip_group_check=True,
                )

            # All front-phase PSUM tiles share the main loop's 3-slot pool so
            # projections, wvthT and g tiles pipeline through the same banks
            # and the PE stays dense from the first DMA to the tail.
            s8 = {}
            g = {}

            def emit_g(mt, gpool):
                gt = gpool.tile([CI, 1024], F32, tag="g", name=f"g{mt}")
                msl = slice(mt * 128, (mt + 1) * 128)
                for h in range(2):
                    nc.tensor.matmul(
                        gt[:, h * 512:(h + 1) * 512],
                        psi_sb[:, msl],
                        phi_sb[:, h * 512:(h + 1) * 512],
                    )
                g[mt] = gt

            def emit_relu(p):
                s = sp.tile([CI, 2, 1024], FP8, tag="s", name=f"s{p}")
                nc.scalar.activation(s[:, 0, :], g.pop(2 * p)[:], AF.Relu)
                nc.vector.tensor_scalar_max(s[:, 1, :], g.pop(2 * p + 1)[:], 0.0)
                s8[p] = s

            def emit_vg(p):
                s = s8.pop(p)
                for h in range(2):
                    nc.tensor.matmul(
                        vg_ps[:, h * 512:(h + 1) * 512],
                        wvth8_sb[:, 2 * p:2 * p + 2, :],
                        s[:, :, h * 512:(h + 1) * 512],
                        start=False, stop=(p == MT // 2 - 1),
                        perf_mode=DR, skip_group_check=True,
                    )

            def emit_proj(c, gpool):
                base = c * 1024
                # psi projection; ScalarE drains with bias
                ps = gpool.tile([CI, 1024], F32, tag="g", name=f"pj{c}")
                for h in range(2):
                    nc.tensor.matmul(
                        ps[0:CO, h * 512:(h + 1) * 512],
                        wpsi_sb,
                        zb16_sb[:, base + h * 512:base + (h + 1) * 512],
                    )
                nc.scalar.activation(
                    psi_sb[:, base:base + 1024], ps[0:CO, :],
                    AF.Identity, bias=bpsi_sb,
                )
                if c == 0:
                    php = gpool.tile([CI, 1024], F32, tag="g", name="phj")
                    for h in range(2):
                        nc.tensor.matmul(
                            php[0:CO, h * 512:(h + 1) * 512],
                            wphi_sb,
                            zb16_sb[:, h * 512:(h + 1) * 512],
                        )
                    nc.vector.tensor_scalar_add(phi_sb[:], php[0:CO, :], bphi_sb)
                # wvthT for the chunk's 8 m-tiles
                wv_ps = gpool.tile([CI, 1024], F32, tag="g", name=f"wv{c}")
                for j in range(8):
                    mt = c * 8 + j
                    nc.tensor.matmul(
                        wv_ps[:, j * 128:(j + 1) * 128],
                        zb16_sb[:, mt * 128:(mt + 1) * 128],
                        wtht_sb,
                        start=(j % 4 == 0), stop=(j % 4 == 3),
                        skip_group_check=True,
                    )
                # + bvth (varies along ci columns): broadcast-tile add
                nc.vector.scalar_tensor_tensor(
                    wvth8_sb[:, c * 8:(c + 1) * 8, :],
                    wv_ps[:],
                    1.0,
                    bvthbc_sb[:],
                    ALU.mult,
                    ALU.add,
                )

            with tc.tile_pool(name="gps", bufs=3, space="PSUM") as gpool:
                for c in range(NCHUNK):
                    emit_proj(c, gpool)
                for p in range(MT // 2):
                    emit_g(2 * p, gpool)
                    emit_g(2 * p + 1, gpool)
                    if p >= 2:
                        emit_vg(p - 2)
                    emit_relu(p)
                emit_vg(MT // 2 - 2)
                emit_vg(MT // 2 - 1)

            # ---- tail: out = vg/4096 + b_v ----
            out0 = tailp.tile([CI, 512], F32, name="out0")
            out1 = tailp.tile([CI, 512], F32, name="out1")
            nc.scalar.activation(
                out0[:], vg_ps[:, 0:512], AF.Identity, bias=bv_sb, scale=SCALE
            )
            nc.vector.tensor_scalar(
                out1[:], vg_ps[:, 512:1024], SCALE, bv_sb, ALU.mult, ALU.add
            )
            nc.sync.dma_start(out[:, 0:512], out0[:])
            nc.sync.dma_start(out[:, 512:1024], out1[:])

    nc.compile()
    return nc


def _get_nc():
    if "nc" not in _CACHE:
        _CACHE["nc"] = _build()
    return _CACHE["nc"]


def build_in_maps(z, w_theta, b_theta, w_phi, b_phi, w_psi, b_psi, w_v, b_v):
    z2 = np.asarray(z, np.float32).reshape(B, CI, N)
    z216 = z2.astype(BF16NP)

    wpsiT = np.asarray(w_psi, np.float32).T          # [128, 64]
    wphiT = np.asarray(w_phi, np.float32).T
    wv = np.asarray(w_v, np.float32)                 # [128, 64]
    wth = wv @ np.asarray(w_theta, np.float32)       # [128, 128]
    wpack = np.ascontiguousarray(
        np.concatenate(
            [wpsiT, wphiT, wth.T, np.float32(N) * np.eye(CI, dtype=np.float32)],
            axis=1,
        ).astype(BF16NP)
    )
    biaspack = np.stack(
        [
            np.concatenate([b_psi, b_phi]).astype(np.float32),
            np.asarray(b_v, np.float32),
            np.zeros(CI, np.float32),
            np.zeros(CI, np.float32),
        ],
        axis=1,
    ).astype(np.float32)
    bvth = wv @ np.asarray(b_theta, np.float32)      # [128]
    bvthbc = np.ascontiguousarray(
        np.broadcast_to(np.tile(bvth, 8), (CI, 1024)).astype(BF16NP)
    )

    in_maps = []
    for core in range(NCORES):
        b, nb = divmod(core, NCORES // B)
        chunks = [nb] + [c for c in range(NCHUNK) if c != nb]
        zperm = np.concatenate(
            [z216[b][:, c * 1024:(c + 1) * 1024] for c in chunks], axis=1
        )
        in_maps.append(
            {
                "wpack": wpack,
                "biaspack": biaspack,
                "bvthbc": bvthbc,
                "zb16": np.ascontiguousarray(zperm),
            }
        )
    return in_maps


def kernel(z, w_theta, b_theta, w_phi, b_phi, w_psi, b_psi, w_v, b_v):
    in_maps = build_in_maps(
        z, w_theta, b_theta, w_phi, b_phi, w_psi, b_psi, w_v, b_v
    )
    nc = _get_nc()
    res = run_bass_kernel_spmd(nc, in_maps, core_ids=list(range(NCORES)))

    out_full = np.empty((B, CI, N), dtype=np.float32)
    for core in range(NCORES):
        b, nb = divmod(core, NCORES // B)
        out_full[b][:, nb * BLK:(nb + 1) * BLK] = res.results[core]["out"]
    return out_full.reshape(B, CI, T, H, W)
